# revision 1
# baseline (speedup 1.0000x reference)
"""Trainium2 Bass kernel for nn_MultiHeadAttention (B=2,S=128,H=16,W=16,E=256, 8 heads).

Sharding: the 512 independent (b,h,w) slices are split 64-per-core across 8
NeuronCores (pure SPMD, no collectives). Each slice is a [S=128, E=256]
self-attention problem.

v2b design (per core, per group of 4 slices):
  - in_proj Q^T as [f,t] matmuls (weights stationary, tokens moving); K^T
    evicted as block-diagonal kbd (rmask multiply during PSUM eviction);
    V in natural [t,c] layout (x^T stationary, weights moving).
  - scores S[q,(j,k)] for 4 heads per matmul via block-diag kbd; causal
    -1000 mask added with a rank-128 mask matmul (start of the group).
  - exp on ACT (PSUM -> SBUF bf16), exact zeros on masked entries;
    all-head transpose to pt[k,(j,q)] via one DMA xbar transpose.
  - row sums Z via accumulating ones-column matmuls into one [32,128]
    PSUM tile per group; one DVE reciprocal per group (NO tensor_reduce,
    NO per-head normalize multiplies).
  - P@V with V stationary (col-packed, 4 heads concurrent) -> o[c,t] PSUM.
  - normalization: indicator matmuls broadcast 1/Z to [c,(sl,t)], then ONE
    fused (o * rb) multiply per group evicts o to SBUF bf16.
  - out_proj bf16 (weights stationary), PSUM -> SBUF -> DRAM via SWDGE.
"""

import os
import sys

import numpy as np

sys.path.insert(0, "/opt/trn_rl_repo")

from contextlib import ExitStack

import concourse.bass as bass
import concourse.mybir as mybir
import concourse.tile as tile
from concourse import bacc
from concourse.bass_utils import run_bass_kernel_spmd

P = 128
NCORES = 8
NSLICE = 64  # slices per core
GSL = 4  # slices per group
NG = NSLICE // GSL  # groups per core
NH = 8
HD = 32
E = 256
S = 128

F32 = mybir.dt.float32
F32R = mybir.dt.float32r
BF16 = mybir.dt.bfloat16
AX = mybir.AxisListType
ALU = mybir.AluOpType
AF = mybir.ActivationFunctionType


def build_program(ng=NG, repeats=1):
    nc = bacc.Bacc("TRN2", target_bir_lowering=False, debug=False, num_devices=NCORES)

    x_d = nc.dram_tensor("x", [ng, 2, P, GSL * S], BF16, kind="ExternalInput").ap()
    wq_d = nc.dram_tensor("wq", [2, P, 256], BF16, kind="ExternalInput").ap()
    wk_d = nc.dram_tensor("wk", [2, P, 256], BF16, kind="ExternalInput").ap()
    wv_d = nc.dram_tensor("wv", [2, P, 256], BF16, kind="ExternalInput").ap()
    wo_d = nc.dram_tensor("wo", [2, P, 256], BF16, kind="ExternalInput").ap()
    am_d = nc.dram_tensor("amask", [P, S], BF16, kind="ExternalInput").ap()
    rm_d = nc.dram_tensor("rmask", [P, 4], F32, kind="ExternalInput").ap()
    ni_d = nc.dram_tensor("negi", [P, 4 * S], BF16, kind="ExternalInput").ap()
    oc_d = nc.dram_tensor("ones32", [P, 32, 32], BF16, kind="ExternalInput").ap()
    ind_d = nc.dram_tensor("ind", [32, 8, P], BF16, kind="ExternalInput").ap()
    y_d = nc.dram_tensor("y", [ng, 2, P, GSL * S], F32, kind="ExternalOutput").ap()

    with tile.TileContext(nc) as tc, ExitStack() as ctx:
        const = ctx.enter_context(tc.tile_pool(name="const", bufs=1))
        wq = const.tile([P, 2, 256], BF16, tag="wq")
        wk = const.tile([P, 2, 256], BF16, tag="wk")
        wv = const.tile([P, 2, 256], BF16, tag="wv")
        wo = const.tile([P, 2, 256], BF16, tag="wo")
        amask = const.tile([P, S], BF16, tag="amask")
        rmask = const.tile([P, 4], F32, tag="rmask")
        negi = const.tile([P, 4 * S], BF16, tag="negi")
        ones32 = const.tile([P, 32, 32], BF16, tag="ones32")
        ind = const.tile([32, 8, P], BF16, tag="ind")
        nc.sync.dma_start(wq[:], wq_d.rearrange("c p f -> p c f"))
        nc.sync.dma_start(wk[:], wk_d.rearrange("c p f -> p c f"))
        nc.sync.dma_start(wv[:], wv_d.rearrange("c p f -> p c f"))
        nc.sync.dma_start(wo[:], wo_d.rearrange("c p f -> p c f"))
        nc.sync.dma_start(amask[:], am_d)
        nc.sync.dma_start(rmask[:], rm_d)
        nc.sync.dma_start(negi[:], ni_d)
        nc.sync.dma_start(ones32[:], oc_d)
        nc.sync.dma_start(ind[:], ind_d)

        xp = ctx.enter_context(tc.tile_pool(name="xp", bufs=3))
        kbp = ctx.enter_context(tc.tile_pool(name="kbp", bufs=2))
        qtp = ctx.enter_context(tc.tile_pool(name="qtp", bufs=2))
        vp = ctx.enter_context(tc.tile_pool(name="vp", bufs=2))
        pnp = ctx.enter_context(tc.tile_pool(name="pnp", bufs=4))
        ptp = ctx.enter_context(tc.tile_pool(name="ptp", bufs=5))
        rcpp = ctx.enter_context(tc.tile_pool(name="rcpp", bufs=2))
        rbp = ctx.enter_context(tc.tile_pool(name="rbp", bufs=2))
        onp = ctx.enter_context(tc.tile_pool(name="onp", bufs=2))
        ysp = ctx.enter_context(tc.tile_pool(name="ysp", bufs=2))

        # PSUM: 3 + 2 + 1 + 2 = 8 banks
        psS = ctx.enter_context(tc.tile_pool(name="psS", bufs=3, space="PSUM"))
        psO = ctx.enter_context(tc.tile_pool(name="psO", bufs=1, space="PSUM"))
        psZ = ctx.enter_context(tc.tile_pool(name="psZ", bufs=1, space="PSUM"))
        psD = ctx.enter_context(tc.tile_pool(name="psD", bufs=2, space="PSUM"))

        def emit_x_load(g):
            x = xp.tile([P, 2, GSL, S], BF16, tag="x")
            nc.sync.dma_start(x[:], x_d[g].rearrange("c p (sl s) -> p c sl s", sl=GSL))
            return x

        def emit_in_proj_kq(x):
            """Q^T [f',ft,sl,t]; block-diag K as kbd[f',ft,jslot,sl,t]."""
            qt = qtp.tile([P, 2, GSL, S], BF16, tag="qt")
            kbd = kbp.tile([P, 2, 4, GSL, S], BF16, tag="kbd")
            for which, wmat in ((0, wq), (1, wk)):
                for ft in range(2):
                    ps = psD.tile([P, GSL * S], F32, tag="d")
                    for ec in range(2):
                        nc.tensor.matmul(
                            ps[:],
                            lhsT=wmat[:, ec, ft * P : (ft + 1) * P],
                            rhs=x[:, ec].rearrange("p a b -> p (a b)"),
                            start=(ec == 0),
                            stop=(ec == 1),
                        )
                    if which == 0:
                        nc.vector.tensor_copy(
                            qt[:, ft].rearrange("p a b -> p (a b)"), ps[:]
                        )
                    else:
                        for jp in range(4):
                            dst = kbd[:, ft, jp].rearrange("p a b -> p (a b)")
                            if jp < 2:
                                nc.scalar.mul(dst, ps[:], rmask[:, jp : jp + 1])
                            else:
                                nc.vector.tensor_scalar(
                                    dst, ps[:], rmask[:, jp : jp + 1], None, ALU.mult
                                )
            return qt, kbd

        def emit_v(x):
            """V: [token 128, sl, c 256] bf16."""
            v = vp.tile([P, GSL, 256], BF16, tag="v")
            for slp in range(GSL // 2):
                psv = psD.tile([P, 2, 256], F32, tag="d")
                for half in range(2):
                    sl = slp * 2 + half
                    for ec in range(2):
                        nc.tensor.matmul(
                            psv[:, half],
                            lhsT=x[:, ec, sl, :],
                            rhs=wv[:, ec, :],
                            start=(ec == 0),
                            stop=(ec == 1),
                        )
                nc.scalar.copy(v[:, slp * 2 : slp * 2 + 2, :], psv[:])
            return v

        def emit_scores_exp(qt, kbd, sl):
            """S[q,(jp,k)] + mask per head group, exp, all-head DMA transpose
            to pt[k,(hg,jp,q)]."""
            pn = pnp.tile([P, 2, GSL, S], BF16, tag="pn")
            pt = ptp.tile([P, 2, GSL, S], BF16, tag="pt")
            for hg in range(2):
                ssc = psS.tile([P, GSL * S], F32, tag="s")
                nc.tensor.matmul(
                    ssc[:],
                    lhsT=amask[:],
                    rhs=negi[:],
                    start=True,
                    stop=False,
                    skip_group_check=True,
                )
                nc.tensor.matmul(
                    ssc[:],
                    lhsT=qt[:, hg, sl, :],
                    rhs=kbd[:, hg, :, sl, :],
                    start=False,
                    stop=True,
                    skip_group_check=True,
                )
                nc.scalar.activation(
                    pn[:, hg].rearrange("p a b -> p (a b)"), ssc[:], AF.Exp
                )
            nc.sync.dma_start_transpose(
                pt[:], pn[:].rearrange("p a b c -> p (a b c)")
            )
            return pt

        def emit_zsum_hg(pt, sl, hg, ztg, first, last):
            """Accumulate per-(sl,head) row sums into ztg [32, 128]."""
            for jp in range(4):
                idx = 8 * sl + 4 * hg + jp
                nc.tensor.matmul(
                    ztg[:],
                    lhsT=ones32[:, idx, :],
                    rhs=pt[:, hg, jp, :],
                    start=(first and jp == 0),
                    stop=(last and jp == 3),
                    skip_group_check=True,
                )

        def emit_pv_hg(v, pt, po_g, sl, hg):
            """o[c', hg, sl, q] = V^T @ P^T (col-packed, 4 heads concurrent)."""
            for jp in range(4):
                j = hg * 4 + jp
                o32 = 32 * jp
                nc.tensor.matmul(
                    po_g[o32 : o32 + 32, hg, sl, :],
                    lhsT=v[:, sl, j * 32 : (j + 1) * 32],
                    rhs=pt[:, hg, jp, :],
                    tile_position=(0, o32),
                )

        def emit_norm(ztg, po_g, g):
            """1/Z, broadcast via indicator matmuls, one fused multiply."""
            rcpT = rcpp.tile([32, S], BF16, tag="rcp")
            with nc.allow_low_precision(reason="softmax denominators fit bf16"):
                nc.vector.reciprocal(rcpT[:], ztg[:])
            rbsb = rbp.tile([P, 2, GSL, S], F32, tag="rb")
            for hg in range(2):
                rb = psD.tile([P, GSL * S], F32, tag="d")
                for sl in range(GSL):
                    nc.tensor.matmul(
                        rb[:, sl * S : (sl + 1) * S],
                        lhsT=ind[:, 2 * sl + hg, :],
                        rhs=rcpT[:],
                        skip_group_check=True,
                    )
                nc.vector.tensor_copy(
                    rbsb[:, hg].rearrange("p a b -> p (a b)"), rb[:]
                )
            on = onp.tile([P, 2, GSL, S], BF16, tag="on")
            nc.vector.scalar_tensor_tensor(
                on[:].rearrange("p a b c -> p (a b c)"),
                po_g[:].rearrange("p a b c -> p (a b c)"),
                1.0,
                rbsb[:].rearrange("p a b c -> p (a b c)"),
                ALU.bypass,
                ALU.mult,
            )
            return on

        def emit_out_proj(on, g):
            y_sb = ysp.tile([P, 2, GSL * S], F32, tag="ysb")
            for et in range(2):
                py = psO.tile([P, GSL * S], F32, tag="po")
                for cc in range(2):
                    nc.tensor.matmul(
                        py[:],
                        lhsT=wo[:, cc, et * P : (et + 1) * P],
                        rhs=on[:, cc].rearrange("p a b -> p (a b)"),
                        start=(cc == 0),
                        stop=(cc == 1),
                    )
                if et == 0:
                    nc.vector.tensor_copy(y_sb[:, et], py[:])
                else:
                    nc.scalar.copy(y_sb[:, et], py[:])
            nc.gpsimd.dma_start(y_d[g].rearrange("e p f -> p e f"), y_sb[:])

        def emit_group_body(g, kqv, kqv_next_emitters):
            """Per-slice chains for group g, interleaved for PE warmth."""
            qt, kbd, v = kqv
            po_g = psO.tile([P, 2, GSL, S], F32, tag="po")
            ztg = psZ.tile([32, S], F32, tag="zt")
            pts = [None] * GSL
            pts[0] = emit_scores_exp(qt, kbd, 0)
            for sl in range(GSL):
                if sl + 1 < GSL:
                    pts[sl + 1] = emit_scores_exp(qt, kbd, sl + 1)
                else:
                    for em in kqv_next_emitters:
                        em()
                for hg in range(2):
                    emit_zsum_hg(pts[sl], sl, hg, ztg,
                                 first=(sl == 0 and hg == 0),
                                 last=(sl == GSL - 1 and hg == 1))
                    emit_pv_hg(v, pts[sl], po_g, sl, hg)
            on = emit_norm(ztg, po_g, g)
            emit_out_proj(on, g)

        for _rep in range(repeats):
            xs = {}
            kqvs = {}
            xs[0] = emit_x_load(0)
            kqvs[0] = (*emit_in_proj_kq(xs[0]), emit_v(xs[0]))
            for g in range(ng):
                if g + 1 < ng:
                    xs[g + 1] = emit_x_load(g + 1)
                    nexts = [
                        lambda g2=g + 1: kqvs.__setitem__(
                            g2, (*emit_in_proj_kq(xs[g2]), emit_v(xs[g2]))
                        )
                    ]
                else:
                    nexts = []
                emit_group_body(g, kqvs[g], nexts)
                del kqvs[g], xs[g]

    nc.compile()
    return nc


def prep_inputs(hidden_state, w_in, w_out):
    """Host-side prep: permute weights per-head, transpose x, shard."""
    import ml_dtypes

    bf16 = ml_dtypes.bfloat16
    B, S_, H, W, E_ = hidden_state.shape
    nsl = B * H * W
    scale = 1.0 / np.sqrt(HD)

    idx_q = np.concatenate([np.arange(i * 96, i * 96 + 32) for i in range(NH)])
    idx_k = idx_q + 32
    idx_v = idx_q + 64
    Wq = (w_in[idx_q] * scale).astype(np.float32)  # [256 f, 256 e]
    Wk = w_in[idx_k].astype(np.float32)
    Wv = w_in[idx_v].astype(np.float32)

    def pack_w(Wm):
        # lhsT layout [ec, ep, f]
        return np.ascontiguousarray(Wm.T.reshape(2, P, 256)).astype(bf16)

    wq_h = pack_w(Wq)
    wk_h = pack_w(Wk)
    wv_h = pack_w(Wv)
    wo_h = np.ascontiguousarray(w_out.T.reshape(2, P, 256)).astype(bf16)

    # mask matmul constants: out[q,(r,s)] = sum_f am[f,q]*negi[f,(r,s)]
    #   = -1000 * am[s,q]  ->  want -1000 iff s(=k) > q  ->  am = tril(ones,-1)
    am_h = np.tril(np.ones((S, S), np.float32), -1).astype(bf16)
    ni_h = np.ascontiguousarray(
        (-1000.0 * np.eye(S, dtype=np.float32))[:, None, :].repeat(4, 1).reshape(S, 4 * S)
    ).astype(bf16)

    # rmask[f', jp] = 1 iff f'//32 == jp (block-diag eviction masks)
    rm_h = np.zeros((P, 4), np.float32)
    for jp in range(4):
        rm_h[32 * jp : 32 * jp + 32, jp] = 1.0

    # ones32[f, idx, m] = 1 iff m == idx (ones-column matrices for row sums)
    oc_h = np.ascontiguousarray(
        np.broadcast_to(np.eye(32, dtype=np.float32)[None], (P, 32, 32))
    ).astype(bf16)

    # ind[p, (sl,hg), c'] = 1 iff p == 8*sl + 4*hg + c'//32
    ind_h = np.zeros((32, 8, P), np.float32)
    for sl in range(GSL):
        for hg in range(2):
            for jp in range(4):
                ind_h[8 * sl + 4 * hg + jp, 2 * sl + hg, 32 * jp : 32 * jp + 32] = 1.0
    ind_h = ind_h.astype(bf16)

    # x^T per slice: [slice, e, s]
    xt = hidden_state.transpose(0, 2, 3, 4, 1).reshape(nsl, E_, S_).astype(bf16)

    in_maps = []
    for c in range(NCORES):
        xs = xt[c * NSLICE : (c + 1) * NSLICE]  # [64, 256, 128]
        xs = xs.reshape(NG, GSL, 2, P, S_).transpose(0, 2, 3, 1, 4)
        xs = np.ascontiguousarray(xs.reshape(NG, 2, P, GSL * S_))
        in_maps.append(
            {
                "x": xs,
                "wq": wq_h,
                "wk": wk_h,
                "wv": wv_h,
                "wo": wo_h,
                "amask": am_h,
                "rmask": rm_h,
                "negi": ni_h,
                "ones32": oc_h,
                "ind": ind_h,
            }
        )
    return in_maps


def assemble_output(results, B=2, H=16, W=16):
    """results: list of 8 dicts with 'y' [NG, 2, 128, GSL*S] f32."""
    ys = []
    for c in range(NCORES):
        y = results[c]["y"].reshape(NG, 2, P, GSL, S)
        y = y.transpose(0, 3, 1, 2, 4).reshape(NSLICE, E, S)
        ys.append(y)
    y_all = np.concatenate(ys, axis=0)  # [512, 256 e, 128 s]
    y_all = y_all.transpose(0, 2, 1)  # [512, s, e]
    out = y_all.reshape(B, H, W, S, E).transpose(0, 3, 1, 2, 4)
    return np.ascontiguousarray(out.astype(np.float32))


_NC_CACHE = {}


def get_program(repeats=1):
    key = repeats
    if key not in _NC_CACHE:
        _NC_CACHE[key] = build_program(repeats=repeats)
    return _NC_CACHE[key]


class _Executor:
    """Cached PJRT executor: builds the shard_map jit once, reuses across calls."""

    def __init__(self, nc):
        import jax
        from jax.sharding import Mesh, PartitionSpec
        from jax.experimental.shard_map import shard_map
        from concourse.bass2jax import _bass_exec_p, install_neuronx_cc_hook, partition_id_tensor

        install_neuronx_cc_hook()
        self.nc = nc
        pname = nc.partition_id_tensor.name if nc.partition_id_tensor else None
        in_names, out_names, out_avals, zero_outs = [], [], [], []
        for alloc in nc.m.functions[0].allocations:
            if not isinstance(alloc, mybir.MemoryLocationSet):
                continue
            name = alloc.memorylocations[0].name
            if alloc.kind == "ExternalInput":
                if name != pname:
                    in_names.append(name)
            elif alloc.kind == "ExternalOutput":
                out_names.append(name)
                shape = tuple(alloc.tensor_shape)
                dtype = mybir.dt.np(alloc.dtype)
                out_avals.append(jax.core.ShapedArray(shape, dtype))
                zero_outs.append(np.zeros(shape, dtype))
        self.in_names = in_names
        self.out_names = out_names
        self.out_avals = out_avals
        n_params = len(in_names)
        all_names = in_names + out_names + ([pname] if pname else [])

        def _body(*args):
            operands = list(args)
            if pname is not None:
                operands.append(partition_id_tensor())
            return tuple(
                _bass_exec_p.bind(
                    *operands,
                    out_avals=tuple(out_avals),
                    in_names=tuple(all_names),
                    out_names=tuple(out_names),
                    lowering_input_output_aliases=(),
                    sim_require_finite=True,
                    sim_require_nnan=True,
                    nc=nc,
                )
            )

        devices = jax.devices()[:NCORES]
        mesh = Mesh(np.asarray(devices), ("core",))
        n_outs = len(out_avals)
        self._jit = jax.jit(
            shard_map(
                _body,
                mesh=mesh,
                in_specs=(PartitionSpec("core"),) * (n_params + n_outs),
                out_specs=(PartitionSpec("core"),) * n_outs,
                check_rep=False,
            ),
            keep_unused=True,
        )
        self._zero_concat = [
            np.zeros((NCORES * z.shape[0], *z.shape[1:]), z.dtype) for z in zero_outs
        ]
        self._jax = jax

    def run(self, in_maps):
        concat_in = [
            np.concatenate([np.asarray(in_maps[c][nm]) for c in range(NCORES)], axis=0)
            for nm in self.in_names
        ]
        outs = self._jit(*concat_in, *self._zero_concat)
        self._jax.block_until_ready(outs)
        return [
            {
                nm: np.asarray(outs[i]).reshape(NCORES, *self.out_avals[i].shape)[c]
                for i, nm in enumerate(self.out_names)
            }
            for c in range(NCORES)
        ]


_EXEC_CACHE = {}


def get_executor(repeats=1):
    if repeats not in _EXEC_CACHE:
        _EXEC_CACHE[repeats] = _Executor(get_program(repeats))
    return _EXEC_CACHE[repeats]


def kernel(hidden_state, w_in, w_out, repeats=1):
    hidden_state = np.asarray(hidden_state, dtype=np.float32)
    w_in = np.asarray(w_in, dtype=np.float32)
    w_out = np.asarray(w_out, dtype=np.float32)
    ex = get_executor(repeats)
    in_maps = prep_inputs(hidden_state, w_in, w_out)
    results = ex.run(in_maps)
    return assemble_output(results)

